# revision 25
# baseline (speedup 1.0000x reference)
"""Conv1d (B=32, C_in=C_out=64, L=16384, K=3, VALID) on 8 trn2 cores.

Strategy: data-parallel over batch (4 batches/core). Each core views its
shard as 2 "pairs" of batches stacked into 128 partitions. The conv is
3 PSUM-accumulated matmuls (one per tap) against a block-diagonal
weight lhsT [128, 128] = diag(W_k^T, W_k^T), so one matmul computes two
batches at full 128-partition PE utilization. Accumulation is fp32 in
PSUM; I/O streams are fp16 to halve HBM traffic (the memory roofline).
Bias is fused into the PSUM->SBUF copy. Shapes hardcoded from the spec.
"""

import os

import numpy as np

from concourse import bacc, bass, mybir, tile
from concourse.bass_utils import run_bass_kernel_spmd

B, C, L, K = 32, 64, 16384, 3
LOUT = L - K + 1  # 16382
NCORES = 8
BPC = B // NCORES  # 4 batches per core
PAIRS = BPC // 2  # 2 stacked pairs per core
P = 128  # partitions (2 x C)
NJ = 512  # PSUM inner chunk (one fp32 bank)

F32 = mybir.dt.float32

# precision mode: f16 I/O (default, ~3e-4 rel err) or f32r / f32
MODE = os.environ.get("CONV_MODE", "f16")
CH = int(os.environ.get("CONV_CH", "4096" if MODE == "f16" else "2048"))
BUFS = int(os.environ.get("CONV_BUFS", "6"))
WARMUP = int(os.environ.get("CONV_WARMUP", "8"))

_NC_CACHE = []


def _io_dtypes():
    if MODE == "f16":
        return mybir.dt.float16, mybir.dt.float16, np.float16
    if MODE == "f32r":
        return mybir.dt.float32r, F32, np.float32
    return F32, F32, np.float32


def _build_nc():
    FIN, FOUT, _ = _io_dtypes()
    nc = bacc.Bacc("TRN2", target_bir_lowering=False, debug=False,
                   num_devices=NCORES)

    x2 = nc.dram_tensor("x2", [PAIRS, P, L], FIN, kind="ExternalInput")
    wT = nc.dram_tensor("wT", [P, K, P], FIN, kind="ExternalInput")
    b2 = nc.dram_tensor("b2", [P, 1], F32, kind="ExternalInput")
    y2 = nc.dram_tensor("y2", [PAIRS, P, LOUT], FOUT, kind="ExternalOutput")

    with tile.TileContext(nc) as tc:
        with (
            tc.tile_pool(name="const", bufs=1) as const_pool,
            tc.tile_pool(name="inp", bufs=BUFS) as inp_pool,
            tc.tile_pool(name="outp", bufs=BUFS) as outp_pool,
            tc.tile_pool(name="psum", bufs=8, space=bass.MemorySpace.PSUM)
            as psum_pool,
        ):
            w = const_pool.tile([P, K, P], FIN)
            nc.sync.dma_start(out=w[:], in_=wT[:])
            bias = const_pool.tile([P, 1], F32)
            nc.sync.dma_start(out=bias[:], in_=b2[:])

            # HAM warm-up: dummy matmuls on zeroed SBUF while the first
            # input DMA is in flight, so the PE clock gate is at 8/8
            # (2.4 GHz) when real work arrives instead of ramping through
            # the first ~3.4us of it.
            if WARMUP:
                wz = const_pool.tile([P, NJ], FIN)
                nc.gpsimd.memset(wz[:], 0.0)
                for i in range(WARMUP):
                    wp = psum_pool.tile([P, NJ], F32, tag="acc",
                                        name=f"warm{i}")
                    nc.tensor.matmul(wp[:], wz[:, :P], wz[:],
                                     start=True, stop=True)

            # Input DMAs issue from Sync (HWDGE, fast first-byte) so the
            # pipeline fills immediately; output DMAs from GpSimd (SWDGE —
            # its slow start overlaps the fill) so an output waiting on
            # drains never head-of-line blocks input prefetch. Chunk sizes
            # are shaped: small first chunk so compute starts early, small
            # last chunks so the compute-gated tail after the final input
            # is short.
            ramp = [512, 1024, 2048]
            tail_small = [CH // 2, 512, 512]
            rest = LOUT - sum(ramp)
            body = [CH] * (rest // CH)
            last = rest - sum(body)
            rest1 = LOUT - sum(tail_small)
            body1 = [CH] * (rest1 // CH)
            last1 = rest1 - sum(body1)
            chunk_lists = {
                0: ramp + body + [last],
                1: body1 + [last1] + tail_small,
            }
            for p in range(PAIRS):
                l0 = 0
                for n in chunk_lists[p % 2]:
                    nin = n + K - 1  # l0 + nin <= L always (LOUT = L-2)
                    it = inp_pool.tile([P, CH + K - 1], FIN, tag="in")
                    nc.sync.dma_start(out=it[:, :nin],
                                      in_=x2[p, :, l0:l0 + nin])
                    ot = outp_pool.tile([P, CH], FOUT, tag="out")
                    for j0 in range(0, n, NJ):
                        nj = min(NJ, n - j0)
                        pt = psum_pool.tile([P, NJ], F32, tag="acc")
                        for k in range(K):
                            nc.tensor.matmul(
                                pt[:, :nj],
                                w[:, k, :],
                                it[:, j0 + k:j0 + k + nj],
                                start=(k == 0),
                                stop=(k == K - 1),
                            )
                        # psum -> sbuf with fused bias add, split across
                        # ACT and DVE so the bank frees twice as fast
                        h = nj // 2
                        nc.scalar.add(ot[:, j0:j0 + h], pt[:, :h],
                                      add=bias[:, 0:1])
                        nc.vector.tensor_scalar_add(ot[:, j0 + h:j0 + nj],
                                                    pt[:, h:nj],
                                                    bias[:, 0:1])
                    nc.gpsimd.dma_start(out=y2[p, :, l0:l0 + n],
                                        in_=ot[:, :n])
                    l0 += n

    nc.compile()
    return nc


def _get_nc():
    if not _NC_CACHE:
        _NC_CACHE.append(_build_nc())
    return _NC_CACHE[0]


def _prep_weights(weight, bias, np_in):
    wT = np.zeros((P, K, P), np.float32)
    for k in range(K):
        wtk = np.ascontiguousarray(weight[:, :, k].T)  # [C_in, C_out]
        wT[0:C, k, 0:C] = wtk
        wT[C:P, k, C:P] = wtk
    b2 = np.concatenate([bias, bias]).reshape(P, 1).astype(np.float32)
    return wT.astype(np_in), b2


def kernel(x, weight, bias, _want_results=False, **run_kwargs):
    x = np.asarray(x, np.float32)
    weight = np.asarray(weight, np.float32)
    bias = np.asarray(bias, np.float32)
    _, _, np_in = _io_dtypes()
    nc = _get_nc()
    wT, b2 = _prep_weights(weight, bias, np_in)
    in_maps = [
        {
            "x2": np.ascontiguousarray(
                x[BPC * i:BPC * (i + 1)].reshape(PAIRS, P, L)).astype(
                    np_in, copy=False),
            "wT": wT,
            "b2": b2,
        }
        for i in range(NCORES)
    ]
    res = run_bass_kernel_spmd(nc, in_maps, list(range(NCORES)), **run_kwargs)
    out = np.concatenate(
        [
            res.results[i]["y2"].astype(np.float32).reshape(BPC, C, LOUT)
            for i in range(NCORES)
        ],
        axis=0,
    )
    if _want_results:
        return out, res
    return out
